# revision 11
# baseline (speedup 1.0000x reference)
"""Trainium2 Bass kernel for nn_MultiHeadModulator (8-core data-parallel over batch).

Math (per reference):
  q/k/v/o projections are F.linear with 4-bit fake-quantized weights + rank-4
  LoRA; attention scores are Re(q conj(k)) per head + relative position bias;
  softmax over past length; real softmax weights applied to complex values;
  output projection.

Strategy:
  - Data-parallel across 8 NeuronCores over the batch dim (B=16 -> 2 per core).
  - Host-side (input prep only): fake-quantize weights to their integer grids
    (bit-exact replication of simq), transpose/pack operands, fold all scalar
    scales into shipped arrays, gather the rel-bias table by position index.
  - Device: all activation compute. bf16 matmuls with fp32 PSUM accumulation.
    Weights are shipped as exact integers in bf16 (values in [-8,7]) so the
    weight side of every projection is exact; quant scales are folded into the
    query / softmax-weight / bias arrays.
  - Contraction dim always on partitions:
      xp   [1024, 4096]   z_past^T slice (token = b_local*2048 + p)
      K    produced as [1024 out, token] chunks, rel-bias added, cast bf16
      V^T  produced as [token, 1024], resident bf16, consumed by PV matmuls
"""

import math
from contextlib import ExitStack

import numpy as np
import ml_dtypes

N_CORES = 8
B, P, DIM, H = 16, 2048, 512, 8
D2 = DIM * 2            # 1024
HD2 = D2 // H           # 128 channels per head ((d, ri) interleaved)
BL = B // N_CORES       # 2 batches per core
NT = P * BL             # 4096 tokens per core
RANK = 4
QMAX = 7
REL_MAX = 64
CH = 8                  # token chunks (contiguous, single batch each)
CT = NT // CH           # 512 tokens per chunk
XT = D2 // 128          # 8 contraction chunks
OT = D2 // 128          # 8 output tiles

BF = ml_dtypes.bfloat16

# test.py introspection: last run's BassKernelResults
LAST_RESULT = None


def _simq_int(W):
    """Exact replication of reference simq(), returning integer grid + scale."""
    W = np.asarray(W, np.float32)
    m = np.max(np.abs(W))
    scale = (m / np.float32(QMAX)).astype(np.float32)
    q = np.clip(np.round(W / (scale + np.float32(1e-8))), -8, 7).astype(np.float32)
    return q, scale


def _simq_full(w):
    q, s = _simq_int(w)
    w = np.asarray(w, np.float32)
    if np.max(np.abs(w)) == 0:
        return w
    return (q * s).astype(np.float32)


def _bf(x):
    return np.ascontiguousarray(np.asarray(x, np.float32).astype(BF))


def _f32(x):
    return np.ascontiguousarray(np.asarray(x, np.float32))


def _prepare(inputs):
    """Host input prep: quantize weights, fold scales, pack layouts.
    Returns (per-core in_maps, so*sv immediate)."""
    z_curr = np.asarray(inputs["z_curr_ri"], np.float32).reshape(B, D2)
    z_past = np.asarray(inputs["z_past_ri"], np.float32).reshape(P, B, D2)
    rel_bias = np.asarray(inputs["rel_bias_ri"], np.float32).reshape(-1, D2)
    curr_pos = int(np.asarray(inputs["curr_pos"]))

    Qk, sk = _simq_int(inputs["Wk"])
    Qv, sv = _simq_int(inputs["Wv"])
    Qq, sq = _simq_int(inputs["Wq"])
    Qo, so = _simq_int(inputs["Wo"])
    bk = _simq_full(inputs["bk"])
    bv = _simq_full(inputs["bv"])
    bq = _simq_full(inputs["bq"])
    bo = _simq_full(inputs["bo"])

    C_q = np.float32(sq * sk / math.sqrt(DIM // H))

    shared = {
        "wk": _bf(Qk.T), "wv": _bf(Qv.T), "wq": _bf(Qq.T), "wo": _bf(Qo.T),
        "ak": _bf(np.asarray(inputs["Ak"], np.float32).T),
        "av": _bf(np.asarray(inputs["Av"], np.float32).T),
        "aq": _bf(np.asarray(inputs["Aq"], np.float32).T),
        "ao": _bf(np.asarray(inputs["Ao"], np.float32).T),
        "lbk": _bf(np.asarray(inputs["LBk"], np.float32).T * np.float32(4.0 / sk)),
        "lbv": _bf(np.asarray(inputs["LBv"], np.float32).T * np.float32(4.0 / sv)),
        "lbq": _bf(np.asarray(inputs["LBq"], np.float32).T * np.float32(4.0 / sq)),
        "lbo": _bf(np.asarray(inputs["LBo"], np.float32).T * np.float32(4.0 / so)),
        "qb": _f32((bq * np.float32(sk / math.sqrt(DIM // H))).reshape(D2, 1)),
        "vb": _f32((so * bv).reshape(D2, 1)),
        "ob": _f32(bo.reshape(D2, 1)),
    }
    idx = np.clip(np.arange(curr_pos - P, curr_pos) + REL_MAX, 0, rel_bias.shape[0] - 1)
    shared["rb"] = _bf((rel_bias[idx].T + bk[:, None]) / sk)  # [D2, P]

    in_maps = []
    for c in range(N_CORES):
        bsl = slice(c * BL, (c + 1) * BL)
        m = dict(shared)
        # [D2, NT], token index t = b_local*P + p
        m["xp"] = _bf(z_past[:, bsl, :].transpose(2, 1, 0).reshape(D2, NT))
        m["xc"] = _bf(z_curr[bsl].T * C_q)  # [D2, BL]
        in_maps.append(m)

    return in_maps, np.float32(so * sv)


def _build(so_sv, debug=False):
    """Build the per-core Bass/Tile program (identical on all 8 cores)."""
    import concourse.bass as bass
    import concourse.tile as tile
    from concourse import bacc, mybir
    from concourse.bass import ts
    from concourse.masks import make_identity

    f32 = mybir.dt.float32
    bf16 = mybir.dt.bfloat16
    PSUM = bass.MemorySpace.PSUM
    Exp = mybir.ActivationFunctionType.Exp
    Copy = mybir.ActivationFunctionType.Copy
    AX = mybir.AxisListType.X
    ADD = mybir.AluOpType.add
    MULT = mybir.AluOpType.mult

    nc = bacc.Bacc("TRN2", target_bir_lowering=False, debug=False,
                   num_devices=N_CORES)

    def din(name, shape, dt=bf16):
        return nc.dram_tensor(name, shape, dt, kind="ExternalInput").ap()

    xp_d = din("xp", [D2, NT])
    xc_d = din("xc", [D2, BL])
    w_d = {n: din("w" + n, [D2, D2]) for n in "kvqo"}
    a_d = {n: din("a" + n, [D2, RANK]) for n in "kvqo"}
    lb_d = {n: din("lb" + n, [RANK, D2]) for n in "kvqo"}
    rb_d = din("rb", [D2, P])
    qb_d = din("qb", [D2, 1], f32)
    vb_d = din("vb", [D2, 1], f32)
    ob_d = din("ob", [D2, 1], f32)
    out_d = nc.dram_tensor("out", [D2, BL], f32, kind="ExternalOutput").ap()
    if debug:
        dbg = {
            "d_qsm": nc.dram_tensor("d_qsm", [128, OT, BL], bf16, kind="ExternalOutput").ap(),
            "d_S": nc.dram_tensor("d_S", [BL * H, P], f32, kind="ExternalOutput").ap(),
            "d_wn": nc.dram_tensor("d_wn", [BL * H, P], bf16, kind="ExternalOutput").ap(),
            "d_rs": nc.dram_tensor("d_rs", [128, OT, BL], bf16, kind="ExternalOutput").ap(),
            "d_kt0": nc.dram_tensor("d_kt0", [128, CT], bf16, kind="ExternalOutput").ap(),
            "d_v0": nc.dram_tensor("d_v0", [128, D2], bf16, kind="ExternalOutput").ap(),
        }

    with tile.TileContext(nc) as tc, ExitStack() as ctx:
        consts = ctx.enter_context(tc.tile_pool(name="consts", bufs=1))
        res = ctx.enter_context(tc.tile_pool(name="res", bufs=1))

        # ---- resident loads ----
        w_s = {}
        for n in "kvqo":
            w_s[n] = consts.tile([128, XT, D2], bf16, tag="w" + n, name="w" + n + "_s")
            nc.sync.dma_start(w_s[n][:], w_d[n].rearrange("(c p) o -> p c o", p=128))
        a_s = {}
        lb_s = {}
        for n in "kvqo":
            a_s[n] = consts.tile([128, XT, RANK], bf16, tag="a" + n, name="a" + n + "_s")
            nc.sync.dma_start(a_s[n][:], a_d[n].rearrange("(c p) r -> p c r", p=128))
            lb_s[n] = consts.tile([RANK, D2], bf16, tag="lb" + n, name="lb" + n + "_s")
            nc.sync.dma_start(lb_s[n][:], lb_d[n][:])
        xc_s = consts.tile([128, XT, BL], bf16, tag="xc")
        nc.sync.dma_start(xc_s[:], xc_d.rearrange("(c p) b -> p c b", p=128))
        qb_s = consts.tile([128, OT, 1], f32, tag="qb")
        nc.sync.dma_start(qb_s[:], qb_d.rearrange("(c p) o -> p c o", p=128))
        vb_s = consts.tile([128, OT, 1], f32, tag="vb")
        nc.sync.dma_start(vb_s[:], vb_d.rearrange("(c p) o -> p c o", p=128))
        ob_s = consts.tile([128, OT, 1], f32, tag="ob")
        nc.sync.dma_start(ob_s[:], ob_d.rearrange("(c p) o -> p c o", p=128))

        ident = consts.tile([BL * H, BL * H], bf16, tag="ident")
        make_identity(nc, ident[:])
        identf = consts.tile([H, H], f32, tag="identf")
        make_identity(nc, identf[:])

        vres = res.tile([128, NT // 128, D2], bf16, tag="vres")
        S = res.tile([BL * H, P], f32, tag="S")
        q_sm = res.tile([128, OT, BL], bf16, tag="q_sm")
        wT = res.tile([128, P // 128, BL * H], bf16, tag="wT")
        r_s = res.tile([128, OT, BL], bf16, tag="r_s")

        # ---- Q projection (tiny) ----
        with tc.tile_pool(name="pq_pool", bufs=2, space=PSUM) as pq_pool:
            t1q_p = pq_pool.tile([RANK, BL], f32, tag="t1q")
            for xc in range(XT):
                nc.tensor.matmul(t1q_p[:], a_s["q"][:, xc, :], xc_s[:, xc, :],
                                 start=(xc == 0), stop=(xc == XT - 1))
            t1q_s = res.tile([RANK, BL], bf16, tag="t1q_s")
            nc.vector.tensor_copy(t1q_s[:], t1q_p[:])
            for ot in range(OT):
                pq = pq_pool.tile([128, BL], f32, tag="pq")
                for xc in range(XT):
                    nc.tensor.matmul(pq[:], w_s["q"][:, xc, ts(ot, 128)],
                                     xc_s[:, xc, :], start=(xc == 0), stop=False)
                nc.tensor.matmul(pq[:], lb_s["q"][:, ts(ot, 128)], t1q_s[:],
                                 start=False, stop=True)
                nc.vector.tensor_scalar(q_sm[:, ot, :], pq[:], qb_s[:, ot, :],
                                        None, ADD)

        # zero-padded per-(batch, head) query slices: q_pad[b][:, h, :] is a
        # [128, 16] lhsT whose only nonzero column is b*H+h (holding
        # q_sm[:, h, b]) -> the per-chunk head matmuls accumulate into
        # disjoint rows of a [16, CT] PSUM tile with zero garbage.
        q_pad = []
        for b in range(BL):
            qp = res.tile([128, H, BL * H], bf16, tag=f"q_pad{b}", name=f"q_pad{b}")
            nc.vector.memset(qp[:], 0.0)
            for h in range(H):
                nc.vector.tensor_copy(qp[:, h, b * H + h:b * H + h + 1],
                                      q_sm[:, h, b:b + 1])
            q_pad.append(qp)

        # ---- main chunk loop: K production + scores + V^T production ----
        with (
            tc.tile_pool(name="xt_pool", bufs=2) as xt_pool,
            tc.tile_pool(name="rb_pool", bufs=2) as rb_pool,
            tc.tile_pool(name="kt_pool", bufs=3) as kt_pool,
            tc.tile_pool(name="t1_pool", bufs=2) as t1_pool,
            tc.tile_pool(name="pk_pool", bufs=2, space=PSUM) as pk_pool,
            tc.tile_pool(name="pv_pool", bufs=2, space=PSUM) as pv_pool,
            tc.tile_pool(name="ps_pool", bufs=2, space=PSUM) as ps_pool,
            tc.tile_pool(name="pt1_pool", bufs=2, space=PSUM) as pt1_pool,
        ):
            for c in range(CH):
                b = c // (CH // BL)          # batch of this chunk
                pc0 = c % (CH // BL)         # position offset, units of CT
                xt = xt_pool.tile([128, XT, CT], bf16, tag="xt")
                nc.sync.dma_start(
                    xt[:], xp_d[:, ts(c, CT)].rearrange("(c p) t -> p c t", p=128))
                rbt = rb_pool.tile([128, OT, CT], bf16, tag="rbt")
                nc.sync.dma_start(
                    rbt[:], rb_d[:, ts(pc0, CT)].rearrange("(c p) t -> p c t", p=128))

                # rank-4 LoRA activations for this chunk
                t1k_p = pt1_pool.tile([RANK, CT], f32, tag="t1")
                for xc in range(XT):
                    nc.tensor.matmul(t1k_p[:], a_s["k"][:, xc, :], xt[:, xc, :],
                                     start=(xc == 0), stop=(xc == XT - 1))
                t1k_s = t1_pool.tile([RANK, CT], bf16, tag="t1k")
                nc.vector.tensor_copy(t1k_s[:], t1k_p[:])

                t1v_p = pt1_pool.tile([RANK, CT], f32, tag="t1")
                for xc in range(XT):
                    nc.tensor.matmul(t1v_p[:], a_s["v"][:, xc, :], xt[:, xc, :],
                                     start=(xc == 0), stop=(xc == XT - 1))
                t1v_s = t1_pool.tile([RANK, CT], bf16, tag="t1v")
                nc.vector.tensor_copy(t1v_s[:], t1v_p[:])

                # K tiles + scores (head h == o-tile index)
                ps_S = ps_pool.tile([BL * H, CT], f32, tag="ps")
                for ot in range(OT):
                    pk = pk_pool.tile([128, CT], f32, tag="pk")
                    for xc in range(XT):
                        nc.tensor.matmul(pk[:], w_s["k"][:, xc, ts(ot, 128)],
                                         xt[:, xc, :], start=(xc == 0), stop=False)
                    nc.tensor.matmul(pk[:], lb_s["k"][:, ts(ot, 128)], t1k_s[:],
                                     start=False, stop=True)
                    kt = kt_pool.tile([128, CT], bf16, tag="kt")
                    nc.vector.tensor_tensor(kt[:], pk[:], rbt[:, ot, :], ADD)
                    nc.tensor.matmul(ps_S[:], q_pad[b][:, ot, :], kt[:],
                                     start=(ot == 0), stop=(ot == OT - 1))
                    if debug and c == 0 and ot == 0:
                        nc.sync.dma_start(dbg["d_kt0"][:], kt[:])

                if c < CH // BL:
                    nc.vector.tensor_copy(S[:, ts(pc0, CT)], ps_S[:])
                else:
                    nc.vector.tensor_tensor(S[:, ts(pc0, CT)], ps_S[:],
                                            S[:, ts(pc0, CT)], ADD)

                # V^T tiles -> resident
                for tt in range(CT // 128):
                    for oh in range(2):
                        pv = pv_pool.tile([128, 512], f32, tag="pv")
                        for xc in range(XT):
                            nc.tensor.matmul(pv[:], xt[:, xc, ts(tt, 128)],
                                             w_s["v"][:, xc, ts(oh, 512)],
                                             start=(xc == 0), stop=False)
                        nc.tensor.matmul(pv[:], t1v_s[:, ts(tt, 128)],
                                         lb_s["v"][:, ts(oh, 512)],
                                         start=False, stop=True)
                        nc.vector.tensor_copy(
                            vres[:, c * (CT // 128) + tt, ts(oh, 512)], pv[:])

        if debug:
            nc.sync.dma_start(dbg["d_qsm"][:], q_sm[:])
            nc.sync.dma_start(dbg["d_S"][:], S[:])
            nc.sync.dma_start(dbg["d_v0"][:], vres[:, 0, :])

        # ---- softmax over the past dimension ----
        with tc.tile_pool(name="ptr_pool", bufs=2, space=PSUM) as ptr_pool:
            mx = res.tile([BL * H, 1], f32, tag="mx")
            nc.vector.reduce_max(mx[:], S[:], axis=AX)
            negmx = res.tile([BL * H, 1], f32, tag="negmx")
            nc.vector.tensor_scalar_mul(negmx[:], mx[:], -1.0)
            E = res.tile([BL * H, P], bf16, tag="E")
            nc.scalar.activation(E[:], S[:], Exp, bias=negmx[:], scale=1.0)
            lsum = res.tile([BL * H, 1], f32, tag="lsum")
            nc.vector.reduce_sum(lsum[:], E[:], axis=AX)
            linv = res.tile([BL * H, 1], f32, tag="linv")
            nc.vector.reciprocal(linv[:], lsum[:])
            wn = res.tile([BL * H, P], bf16, tag="wn")
            # fold so*sv into the softmax weights
            nc.vector.tensor_scalar(wn[:], E[:], linv[:], float(so_sv),
                                    MULT, MULT)
            # transpose [BL*H, P] -> [P, BL*H] in 128-column blocks
            for pc in range(P // 128):
                ptr = ptr_pool.tile([128, BL * H], bf16, tag="ptr")
                nc.tensor.transpose(ptr[:], wn[:, ts(pc, 128)], ident[:])
                nc.vector.tensor_copy(wT[:, pc, :], ptr[:])

        if debug:
            nc.sync.dma_start(dbg["d_wn"][:], wn[:])

        # ---- PV + output-side assembly (R = so*sv*PV + so*bv, [din, b]) ----
        with (
            tc.tile_pool(name="pacc_pool", bufs=4, space=PSUM) as pacc_pool,
            tc.tile_pool(name="ptr2_pool", bufs=2, space=PSUM) as ptr2_pool,
            tc.tile_pool(name="pvs_pool", bufs=2) as pvs_pool,
        ):
            for b in range(BL):
                for oh in range(2):
                    pacc = pacc_pool.tile([H, 512], f32, tag="pacc")
                    for pc in range(P // 128):
                        nc.tensor.matmul(pacc[:],
                                         wT[:, pc, b * H:(b + 1) * H],
                                         vres[:, b * (P // 128) + pc, ts(oh, 512)],
                                         start=(pc == 0),
                                         stop=(pc == P // 128 - 1))
                    pvs = pvs_pool.tile([H, 512], f32, tag="pvs")
                    nc.vector.tensor_copy(pvs[:], pacc[:])
                    for hl in range(4):
                        h = oh * 4 + hl
                        ptr2 = ptr2_pool.tile([128, H], f32, tag="ptr2")
                        nc.tensor.transpose(ptr2[:], pvs[:, ts(hl, 128)], identf[:])
                        nc.vector.tensor_scalar(r_s[:, h, b:b + 1],
                                                ptr2[:, h:h + 1],
                                                vb_s[:, h, :], None, ADD)

        if debug:
            nc.sync.dma_start(dbg["d_rs"][:], r_s[:])

        # ---- O projection ----
        with (
            tc.tile_pool(name="po_pool", bufs=2, space=PSUM) as po_pool,
            tc.tile_pool(name="pt1o_pool", bufs=1, space=PSUM) as pt1o_pool,
            tc.tile_pool(name="out_pool", bufs=2) as out_pool,
        ):
            t1o_p = pt1o_pool.tile([RANK, BL], f32, tag="t1o")
            for xc in range(XT):
                nc.tensor.matmul(t1o_p[:], a_s["o"][:, xc, :], r_s[:, xc, :],
                                 start=(xc == 0), stop=(xc == XT - 1))
            t1o_s = res.tile([RANK, BL], bf16, tag="t1o_s")
            nc.vector.tensor_copy(t1o_s[:], t1o_p[:])
            for ot in range(OT):
                po = po_pool.tile([128, BL], f32, tag="po")
                for xc in range(XT):
                    nc.tensor.matmul(po[:], w_s["o"][:, xc, ts(ot, 128)],
                                     r_s[:, xc, :], start=(xc == 0), stop=False)
                nc.tensor.matmul(po[:], lb_s["o"][:, ts(ot, 128)], t1o_s[:],
                                 start=False, stop=True)
                ot_s = out_pool.tile([128, BL], f32, tag="ot_s")
                nc.vector.tensor_scalar(ot_s[:], po[:], ob_s[:, ot, :], None, ADD)
                nc.sync.dma_start(out_d[ts(ot, 128), :], ot_s[:])

    nc.compile()
    return nc


def kernel(**inputs):
    global LAST_RESULT
    from concourse.bass_utils import run_bass_kernel_spmd

    in_maps, so_sv = _prepare(inputs)
    nc = _build(so_sv)
    res = run_bass_kernel_spmd(nc, in_maps, list(range(N_CORES)))
    LAST_RESULT = res
    outs = []
    for c in range(N_CORES):
        o = np.asarray(res.results[c]["out"], np.float32)  # [D2, BL]
        outs.append(o.T.reshape(BL, DIM, 2))
    return np.concatenate(outs, axis=0).astype(np.float32)


# revision 14
# speedup vs baseline: 1.1152x; 1.1152x over previous
"""Trainium2 Bass kernel for nn_MultiHeadModulator (8-core data-parallel over batch).

Math (per reference):
  q/k/v/o projections are F.linear with 4-bit fake-quantized weights + rank-4
  LoRA; attention scores are Re(q conj(k)) per head + relative position bias;
  softmax over past length; real softmax weights applied to complex values;
  output projection.

Strategy:
  - Data-parallel across 8 NeuronCores over the batch dim (B=16 -> 2 per core).
  - Host-side (input prep only): fake-quantize weights to their integer grids
    (bit-exact replication of simq), transpose/pack operands, fold all scalar
    scales into shipped arrays, gather the rel-bias table by position index.
  - Device: all activation compute. bf16 matmuls with fp32 PSUM accumulation.
    Weights are shipped as exact integers in bf16 (values in [-8,7]) so the
    weight side of every projection is exact; quant scales are folded into the
    query / softmax-weight / bias arrays.
  - Contraction dim always on partitions:
      xp   [1024, 4096]   z_past^T slice (token = b_local*2048 + p)
      K    produced as [1024 out, token] chunks, rel-bias added, cast bf16
      V^T  produced as [token, 1024], resident bf16, consumed by PV matmuls
"""

import math
from contextlib import ExitStack

import numpy as np
import ml_dtypes

N_CORES = 8
B, P, DIM, H = 16, 2048, 512, 8
D2 = DIM * 2            # 1024
HD2 = D2 // H           # 128 channels per head ((d, ri) interleaved)
BL = B // N_CORES       # 2 batches per core
NT = P * BL             # 4096 tokens per core
RANK = 4
QMAX = 7
REL_MAX = 64
CH = 8                  # token chunks (contiguous, single batch each)
CT = NT // CH           # 512 tokens per chunk
XT = D2 // 128          # 8 contraction chunks
OT = D2 // 128          # 8 output tiles

BF = ml_dtypes.bfloat16

# test.py introspection: last run's BassKernelResults
LAST_RESULT = None


def _simq_int(W):
    """Exact replication of reference simq(), returning integer grid + scale."""
    W = np.asarray(W, np.float32)
    m = np.max(np.abs(W))
    scale = (m / np.float32(QMAX)).astype(np.float32)
    q = np.clip(np.round(W / (scale + np.float32(1e-8))), -8, 7).astype(np.float32)
    return q, scale


def _simq_full(w):
    q, s = _simq_int(w)
    w = np.asarray(w, np.float32)
    if np.max(np.abs(w)) == 0:
        return w
    return (q * s).astype(np.float32)


def _bf(x):
    return np.ascontiguousarray(np.asarray(x, np.float32).astype(BF))


def _f32(x):
    return np.ascontiguousarray(np.asarray(x, np.float32))


def _prepare(inputs):
    """Host input prep: quantize weights, fold scales, pack layouts.
    Returns (per-core in_maps, so*sv immediate)."""
    z_curr = np.asarray(inputs["z_curr_ri"], np.float32).reshape(B, D2)
    z_past = np.asarray(inputs["z_past_ri"], np.float32).reshape(P, B, D2)
    rel_bias = np.asarray(inputs["rel_bias_ri"], np.float32).reshape(-1, D2)
    curr_pos = int(np.asarray(inputs["curr_pos"]))

    Qk, sk = _simq_int(inputs["Wk"])
    Qv, sv = _simq_int(inputs["Wv"])
    Qq, sq = _simq_int(inputs["Wq"])
    Qo, so = _simq_int(inputs["Wo"])
    bk = _simq_full(inputs["bk"])
    bv = _simq_full(inputs["bv"])
    bq = _simq_full(inputs["bq"])
    bo = _simq_full(inputs["bo"])

    C_q = np.float32(sq * sk / math.sqrt(DIM // H))

    shared = {
        "wk": _bf(Qk.T), "wv": _bf(Qv.T), "wq": _bf(Qq.T), "wo": _bf(Qo.T),
        "aq": _bf(np.asarray(inputs["Aq"], np.float32).T),
        "ao": _bf(np.asarray(inputs["Ao"], np.float32).T),
        "lbq": _bf(np.asarray(inputs["LBq"], np.float32).T * np.float32(4.0 / sq)),
        "lbo": _bf(np.asarray(inputs["LBo"], np.float32).T * np.float32(4.0 / so)),
        "qb": _f32((bq * np.float32(sk / math.sqrt(DIM // H))).reshape(D2, 1)),
        "vb": _f32((so * bv).reshape(D2, 1)),
        "ob": _f32(bo.reshape(D2, 1)),
    }
    # merged K/V rank-4 operands: akv = [Ak^T | Av^T]; lb*8 padded to 8 rows so
    # all contractions against the shared [8, CT] t1kv activations are legal
    lbk_e = np.asarray(inputs["LBk"], np.float32).T * np.float32(4.0 / sk)  # [4, D2]
    lbv_e = np.asarray(inputs["LBv"], np.float32).T * np.float32(4.0 / sv)
    shared["akv"] = _bf(np.concatenate(
        [np.asarray(inputs["Ak"], np.float32).T,
         np.asarray(inputs["Av"], np.float32).T], axis=1))          # [D2, 8]
    lbv8 = np.zeros((2 * RANK, D2), np.float32); lbv8[RANK:] = lbv_e
    shared["lbv8"] = _bf(lbv8)
    shared["lbkT"] = _bf(lbk_e.T)                                    # [D2, 4]
    idx = np.clip(np.arange(curr_pos - P, curr_pos) + REL_MAX, 0, rel_bias.shape[0] - 1)
    shared["rb"] = _bf((rel_bias[idx].T + bk[:, None]) / sk)  # [D2, P]

    in_maps = []
    for c in range(N_CORES):
        bsl = slice(c * BL, (c + 1) * BL)
        m = dict(shared)
        # [D2, NT], token index t = b_local*P + p
        m["xp"] = _bf(z_past[:, bsl, :].transpose(2, 1, 0).reshape(D2, NT))
        m["xc"] = _bf(z_curr[bsl].T * C_q)  # [D2, BL]
        in_maps.append(m)

    return in_maps, np.float32(so * sv)


def _build(so_sv, debug=False):
    """Build the per-core Bass/Tile program (identical on all 8 cores)."""
    import concourse.bass as bass
    import concourse.tile as tile
    from concourse import bacc, mybir
    from concourse.bass import ts
    from concourse.masks import make_identity

    f32 = mybir.dt.float32
    bf16 = mybir.dt.bfloat16
    PSUM = bass.MemorySpace.PSUM
    Exp = mybir.ActivationFunctionType.Exp
    Copy = mybir.ActivationFunctionType.Copy
    AX = mybir.AxisListType.X
    ADD = mybir.AluOpType.add
    MULT = mybir.AluOpType.mult

    nc = bacc.Bacc("TRN2", target_bir_lowering=False, debug=False,
                   num_devices=N_CORES)

    def din(name, shape, dt=bf16):
        return nc.dram_tensor(name, shape, dt, kind="ExternalInput").ap()

    xp_d = din("xp", [D2, NT])
    xc_d = din("xc", [D2, BL])
    w_d = {n: din("w" + n, [D2, D2]) for n in "kvqo"}
    a_d = {n: din("a" + n, [D2, RANK]) for n in "qo"}
    lb_d = {n: din("lb" + n, [RANK, D2]) for n in "qo"}
    akv_d = din("akv", [D2, 2 * RANK])
    lbv8_d = din("lbv8", [2 * RANK, D2])
    lbkT_d = din("lbkT", [D2, RANK])
    rb_d = din("rb", [D2, P])
    qb_d = din("qb", [D2, 1], f32)
    vb_d = din("vb", [D2, 1], f32)
    ob_d = din("ob", [D2, 1], f32)
    out_d = nc.dram_tensor("out", [D2, BL], f32, kind="ExternalOutput").ap()
    if debug:
        dbg = {
            "d_qsm": nc.dram_tensor("d_qsm", [128, OT, BL], bf16, kind="ExternalOutput").ap(),
            "d_S": nc.dram_tensor("d_S", [BL * H, P], f32, kind="ExternalOutput").ap(),
            "d_wn": nc.dram_tensor("d_wn", [BL * H, P], bf16, kind="ExternalOutput").ap(),
            "d_rs": nc.dram_tensor("d_rs", [128, OT, BL], bf16, kind="ExternalOutput").ap(),
            "d_kt0": nc.dram_tensor("d_kt0", [128, CT], bf16, kind="ExternalOutput").ap(),
            "d_v0": nc.dram_tensor("d_v0", [128, D2], bf16, kind="ExternalOutput").ap(),
        }

    with tile.TileContext(nc) as tc, ExitStack() as ctx:
        consts = ctx.enter_context(tc.tile_pool(name="consts", bufs=1))
        res = ctx.enter_context(tc.tile_pool(name="res", bufs=1))

        # ---- resident loads (DMA emission order = startup critical path:
        # Q-phase deps first, then chunk-0 K deps, then V, then O-side) ----
        w_s = {}
        a_s = {}
        lb_s = {}

        def wload(n):
            w_s[n] = consts.tile([128, XT, D2], bf16, tag="w" + n, name="w" + n + "_s")
            nc.sync.dma_start(w_s[n][:], w_d[n].rearrange("(c p) o -> p c o", p=128))

        def ablload(n):
            a_s[n] = consts.tile([128, XT, RANK], bf16, tag="a" + n, name="a" + n + "_s")
            nc.sync.dma_start(a_s[n][:], a_d[n].rearrange("(c p) r -> p c r", p=128))
            lb_s[n] = consts.tile([RANK, D2], bf16, tag="lb" + n, name="lb" + n + "_s")
            nc.sync.dma_start(lb_s[n][:], lb_d[n][:])

        xc_s = consts.tile([128, XT, BL], bf16, tag="xc")
        nc.sync.dma_start(xc_s[:], xc_d.rearrange("(c p) b -> p c b", p=128))
        qb_s = consts.tile([128, OT, 1], f32, tag="qb")
        nc.sync.dma_start(qb_s[:], qb_d.rearrange("(c p) o -> p c o", p=128))
        ablload("q")
        lbkT_s = consts.tile([128, OT, RANK], bf16, tag="lbkT")
        nc.sync.dma_start(lbkT_s[:], lbkT_d.rearrange("(c p) r -> p c r", p=128))
        wload("q")
        akv_s = consts.tile([128, XT, 2 * RANK], bf16, tag="akv")
        nc.sync.dma_start(akv_s[:], akv_d.rearrange("(c p) r -> p c r", p=128))
        wload("k")
        lbv8_s = consts.tile([2 * RANK, D2], bf16, tag="lbv8")
        nc.sync.dma_start(lbv8_s[:], lbv8_d[:])
        wload("v")
        wload("o")
        ablload("o")
        vb_s = consts.tile([128, OT, 1], f32, tag="vb")
        nc.sync.dma_start(vb_s[:], vb_d.rearrange("(c p) o -> p c o", p=128))
        ob_s = consts.tile([128, OT, 1], f32, tag="ob")
        nc.sync.dma_start(ob_s[:], ob_d.rearrange("(c p) o -> p c o", p=128))

        ident = consts.tile([BL * H, BL * H], bf16, tag="ident")
        make_identity(nc, ident[:])
        identf = consts.tile([H, H], f32, tag="identf")
        make_identity(nc, identf[:])

        vres = res.tile([128, NT // 128, D2], bf16, tag="vres")
        S = res.tile([BL * H, P], f32, tag="S")
        q_sm = res.tile([128, OT, BL], bf16, tag="q_sm")
        wT = res.tile([128, P // 128, BL * H], bf16, tag="wT")
        r_s = res.tile([128, OT, BL], bf16, tag="r_s")

        # ---- Q projection (tiny) ----
        with tc.tile_pool(name="pq_pool", bufs=2, space=PSUM) as pq_pool:
            t1q_p = pq_pool.tile([RANK, BL], f32, tag="t1q")
            for xc in range(XT):
                nc.tensor.matmul(t1q_p[:], a_s["q"][:, xc, :], xc_s[:, xc, :],
                                 start=(xc == 0), stop=(xc == XT - 1))
            t1q_s = res.tile([RANK, BL], bf16, tag="t1q_s")
            nc.vector.tensor_copy(t1q_s[:], t1q_p[:])
            for ot in range(OT):
                pq = pq_pool.tile([128, BL], f32, tag="pq")
                for xc in range(XT):
                    nc.tensor.matmul(pq[:], w_s["q"][:, xc, ts(ot, 128)],
                                     xc_s[:, xc, :], start=(xc == 0), stop=False)
                nc.tensor.matmul(pq[:], lb_s["q"][:, ts(ot, 128)], t1q_s[:],
                                 start=False, stop=True)
                nc.vector.tensor_scalar(q_sm[:, ot, :], pq[:], qb_s[:, ot, :],
                                        None, ADD)

        # zero-padded per-(batch, head) query slices: q_pad[b][:, h, :] is a
        # [128, 16] lhsT whose only nonzero column is b*H+h (holding
        # q_sm[:, h, b]) -> the per-chunk head matmuls accumulate into
        # disjoint rows of a [16, CT] PSUM tile with zero garbage.
        q_pad = []
        for b in range(BL):
            qp = res.tile([128, H, BL * H], bf16, tag=f"q_pad{b}", name=f"q_pad{b}")
            nc.vector.memset(qp[:], 0.0)
            for h in range(H):
                nc.vector.tensor_copy(qp[:, h, b * H + h:b * H + h + 1],
                                      q_sm[:, h, b:b + 1])
            q_pad.append(qp)

        # qlb[b, h, r] = sum_o q_sm[o, b] * lbkT[o, r] over head-block h; packed
        # zero-padded into [2*RANK, 16] lhsT tiles (rows RANK..2R zero) so the
        # K-side LoRA score term is a single rank-contraction matmul per chunk
        # against the shared t1kv activations.
        qlb_pad = []
        for b in range(BL):
            qlp = res.tile([2 * RANK, BL * H], bf16, tag=f"qlb_pad{b}",
                           name=f"qlb_pad{b}")
            nc.vector.memset(qlp[:], 0.0)
            qlb_pad.append(qlp)
        with tc.tile_pool(name="pqlb_pool", bufs=2, space=PSUM) as pqlb_pool:
            for h in range(H):
                pqlb = pqlb_pool.tile([BL, RANK], f32, tag="pqlb")
                nc.tensor.matmul(pqlb[:], q_sm[:, h, :], lbkT_s[:, h, :],
                                 start=True, stop=True)
                qlb_sb = res.tile([BL, RANK], bf16, tag="qlb_sb")
                nc.vector.tensor_copy(qlb_sb[:], pqlb[:])
                pqt = pqlb_pool.tile([RANK, BL], bf16, tag="pqt")
                nc.tensor.transpose(pqt[:], qlb_sb[:], ident[:BL, :BL])
                for b in range(BL):
                    nc.vector.tensor_copy(
                        qlb_pad[b][0:RANK, b * H + h:b * H + h + 1],
                        pqt[:, b:b + 1])

        # ---- main chunk loop: K production + scores + V^T production ----
        with (
            tc.tile_pool(name="xt_pool", bufs=2) as xt_pool,
            tc.tile_pool(name="rb_pool", bufs=2) as rb_pool,
            tc.tile_pool(name="kt_pool", bufs=3) as kt_pool,
            tc.tile_pool(name="t1_pool", bufs=2) as t1_pool,
            tc.tile_pool(name="pkv_pool", bufs=5, space=PSUM) as pkv_pool,
            tc.tile_pool(name="ps_pool", bufs=2, space=PSUM) as ps_pool,
            tc.tile_pool(name="pt1_pool", bufs=1, space=PSUM) as pt1_pool,
        ):
            for c in range(CH):
                b = c // (CH // BL)          # batch of this chunk
                pc0 = c % (CH // BL)         # position offset, units of CT
                xt = xt_pool.tile([128, XT, CT], bf16, tag="xt")
                nc.sync.dma_start(
                    xt[:], xp_d[:, ts(c, CT)].rearrange("(c p) t -> p c t", p=128))
                rbt = rb_pool.tile([128, OT, CT], bf16, tag="rbt")
                nc.sync.dma_start(
                    rbt[:], rb_d[:, ts(pc0, CT)].rearrange("(c p) t -> p c t", p=128))

                # merged K/V rank-4 LoRA activations for this chunk
                t1kv_p = pt1_pool.tile([2 * RANK, CT], f32, tag="t1")
                for xc in range(XT):
                    nc.tensor.matmul(t1kv_p[:], akv_s[:, xc, :], xt[:, xc, :],
                                     start=(xc == 0), stop=(xc == XT - 1))
                t1kv_s = t1_pool.tile([2 * RANK, CT], bf16, tag="t1kv")
                nc.vector.tensor_copy(t1kv_s[:], t1kv_p[:])

                # scores PSUM accumulator; first the K-LoRA term, then per-head
                # dot products, software-pipelined one head behind K production
                # so the rel-bias add (DVE) never stalls the PE.
                ps_S = ps_pool.tile([BL * H, CT], f32, tag="ps")
                nc.tensor.matmul(ps_S[:], qlb_pad[b][:], t1kv_s[:],
                                 start=True, stop=False)
                kts = [None] * OT
                for ot in range(OT):
                    pk = pkv_pool.tile([128, CT], f32, tag="pkv", name="pk")
                    for xc in range(XT):
                        nc.tensor.matmul(pk[:], w_s["k"][:, xc, ts(ot, 128)],
                                         xt[:, xc, :], start=(xc == 0),
                                         stop=(xc == XT - 1))
                    kt = kt_pool.tile([128, CT], bf16, tag="kt")
                    nc.vector.tensor_tensor(kt[:], pk[:], rbt[:, ot, :], ADD)
                    kts[ot] = kt
                    if ot > 0:
                        nc.tensor.matmul(ps_S[:], q_pad[b][:, ot - 1, :],
                                         kts[ot - 1][:], start=False, stop=False)
                    if debug and c == 0 and ot == 0:
                        nc.sync.dma_start(dbg["d_kt0"][:], kt[:])
                nc.tensor.matmul(ps_S[:], q_pad[b][:, OT - 1, :],
                                 kts[OT - 1][:], start=False, stop=True)

                if c < CH // BL:
                    nc.vector.tensor_copy(S[:, ts(pc0, CT)], ps_S[:])
                else:
                    nc.vector.tensor_tensor(S[:, ts(pc0, CT)], ps_S[:],
                                            S[:, ts(pc0, CT)], ADD)

                # V^T tiles -> resident
                for tt in range(CT // 128):
                    for oh in range(2):
                        pv = pkv_pool.tile([128, 512], f32, tag="pkv", name="pv")
                        for xc in range(XT):
                            nc.tensor.matmul(pv[:], xt[:, xc, ts(tt, 128)],
                                             w_s["v"][:, xc, ts(oh, 512)],
                                             start=(xc == 0), stop=False)
                        nc.tensor.matmul(pv[:], t1kv_s[:, ts(tt, 128)],
                                         lbv8_s[:, ts(oh, 512)],
                                         start=False, stop=True)
                        nc.vector.tensor_copy(
                            vres[:, c * (CT // 128) + tt, ts(oh, 512)], pv[:])

        if debug:
            nc.sync.dma_start(dbg["d_qsm"][:], q_sm[:])
            nc.sync.dma_start(dbg["d_S"][:], S[:])
            nc.sync.dma_start(dbg["d_v0"][:], vres[:, 0, :])

        # ---- softmax over the past dimension ----
        with tc.tile_pool(name="ptr_pool", bufs=2, space=PSUM) as ptr_pool:
            mx = res.tile([BL * H, 1], f32, tag="mx")
            nc.vector.reduce_max(mx[:], S[:], axis=AX)
            negmx = res.tile([BL * H, 1], f32, tag="negmx")
            nc.vector.tensor_scalar_mul(negmx[:], mx[:], -1.0)
            E = res.tile([BL * H, P], bf16, tag="E")
            nc.scalar.activation(E[:], S[:], Exp, bias=negmx[:], scale=1.0)
            lsum = res.tile([BL * H, 1], f32, tag="lsum")
            nc.vector.reduce_sum(lsum[:], E[:], axis=AX)
            linv = res.tile([BL * H, 1], f32, tag="linv")
            nc.vector.reciprocal(linv[:], lsum[:])
            wn = res.tile([BL * H, P], bf16, tag="wn")
            # fold so*sv into the softmax weights
            nc.vector.tensor_scalar(wn[:], E[:], linv[:], float(so_sv),
                                    MULT, MULT)
            # transpose [BL*H, P] -> [P, BL*H] in 128-column blocks
            for pc in range(P // 128):
                ptr = ptr_pool.tile([128, BL * H], bf16, tag="ptr")
                nc.tensor.transpose(ptr[:], wn[:, ts(pc, 128)], ident[:])
                nc.vector.tensor_copy(wT[:, pc, :], ptr[:])

        if debug:
            nc.sync.dma_start(dbg["d_wn"][:], wn[:])

        # ---- PV + output-side assembly (R = so*sv*PV + so*bv, [din, b]) ----
        with (
            tc.tile_pool(name="pacc_pool", bufs=4, space=PSUM) as pacc_pool,
            tc.tile_pool(name="ptr2_pool", bufs=2, space=PSUM) as ptr2_pool,
            tc.tile_pool(name="pvs_pool", bufs=2) as pvs_pool,
        ):
            for b in range(BL):
                for oh in range(2):
                    pacc = pacc_pool.tile([H, 512], f32, tag="pacc")
                    for pc in range(P // 128):
                        nc.tensor.matmul(pacc[:],
                                         wT[:, pc, b * H:(b + 1) * H],
                                         vres[:, b * (P // 128) + pc, ts(oh, 512)],
                                         start=(pc == 0),
                                         stop=(pc == P // 128 - 1))
                    pvs = pvs_pool.tile([H, 512], f32, tag="pvs")
                    nc.vector.tensor_copy(pvs[:], pacc[:])
                    for hl in range(4):
                        h = oh * 4 + hl
                        ptr2 = ptr2_pool.tile([128, H], f32, tag="ptr2")
                        nc.tensor.transpose(ptr2[:], pvs[:, ts(hl, 128)], identf[:])
                        nc.vector.tensor_scalar(r_s[:, h, b:b + 1],
                                                ptr2[:, h:h + 1],
                                                vb_s[:, h, :], None, ADD)

        if debug:
            nc.sync.dma_start(dbg["d_rs"][:], r_s[:])

        # ---- O projection ----
        with (
            tc.tile_pool(name="po_pool", bufs=2, space=PSUM) as po_pool,
            tc.tile_pool(name="pt1o_pool", bufs=1, space=PSUM) as pt1o_pool,
            tc.tile_pool(name="out_pool", bufs=2) as out_pool,
        ):
            t1o_p = pt1o_pool.tile([RANK, BL], f32, tag="t1o")
            for xc in range(XT):
                nc.tensor.matmul(t1o_p[:], a_s["o"][:, xc, :], r_s[:, xc, :],
                                 start=(xc == 0), stop=(xc == XT - 1))
            t1o_s = res.tile([RANK, BL], bf16, tag="t1o_s")
            nc.vector.tensor_copy(t1o_s[:], t1o_p[:])
            for ot in range(OT):
                po = po_pool.tile([128, BL], f32, tag="po")
                for xc in range(XT):
                    nc.tensor.matmul(po[:], w_s["o"][:, xc, ts(ot, 128)],
                                     r_s[:, xc, :], start=(xc == 0), stop=False)
                nc.tensor.matmul(po[:], lb_s["o"][:, ts(ot, 128)], t1o_s[:],
                                 start=False, stop=True)
                ot_s = out_pool.tile([128, BL], f32, tag="ot_s")
                nc.vector.tensor_scalar(ot_s[:], po[:], ob_s[:, ot, :], None, ADD)
                nc.sync.dma_start(out_d[ts(ot, 128), :], ot_s[:])

    nc.compile()
    return nc


def kernel(**inputs):
    global LAST_RESULT
    from concourse.bass_utils import run_bass_kernel_spmd

    in_maps, so_sv = _prepare(inputs)
    nc = _build(so_sv)
    res = run_bass_kernel_spmd(nc, in_maps, list(range(N_CORES)))
    LAST_RESULT = res
    outs = []
    for c in range(N_CORES):
        o = np.asarray(res.results[c]["out"], np.float32)  # [D2, BL]
        outs.append(o.T.reshape(BL, DIM, 2))
    return np.concatenate(outs, axis=0).astype(np.float32)


# revision 15
# speedup vs baseline: 1.1553x; 1.0360x over previous
"""Trainium2 Bass kernel for nn_MultiHeadModulator (8-core data-parallel over batch).

Math (per reference):
  q/k/v/o projections are F.linear with 4-bit fake-quantized weights + rank-4
  LoRA; attention scores are Re(q conj(k)) per head + relative position bias;
  softmax over past length; real softmax weights applied to complex values;
  output projection.

Strategy:
  - Data-parallel across 8 NeuronCores over the batch dim (B=16 -> 2 per core).
  - Host-side (input prep only): fake-quantize weights to their integer grids
    (bit-exact replication of simq), transpose/pack operands into
    partition-major layouts (contiguous per SBUF partition, so every DMA is
    128 large descriptors), fold all scalar scales into shipped arrays, gather
    the rel-bias table by position index.
  - Device: all activation compute. bf16 matmuls with fp32 PSUM accumulation.
    Weights are shipped as exact integers in bf16 (values in [-8,7]) so the
    weight side of every projection is exact; quant scales are folded into the
    query / softmax-weight / bias arrays.
  - Contraction dim always on partitions; LoRA rank contractions are padded to
    128 rows with zeros so the PE's background weight-load pipelining never
    breaks; per-head score matmuls use one-hot-column query tiles accumulating
    into a per-chunk [8, CT] PSUM tile.
"""

import math
from contextlib import ExitStack

import numpy as np
import ml_dtypes

N_CORES = 8
B, P, DIM, H = 16, 2048, 512, 8
D2 = DIM * 2            # 1024
HD2 = D2 // H           # 128 channels per head ((d, ri) interleaved)
BL = B // N_CORES       # 2 batches per core
NT = P * BL             # 4096 tokens per core
RANK = 4
QMAX = 7
REL_MAX = 64
CH = 8                  # token chunks (contiguous, single batch each)
CT = NT // CH           # 512 tokens per chunk
XT = D2 // 128          # 8 contraction chunks
OT = D2 // 128          # 8 output tiles
PC = P // 128           # 16 position chunks per batch

BF = ml_dtypes.bfloat16

# test.py introspection: last run's BassKernelResults
LAST_RESULT = None


def _simq_int(W):
    """Exact replication of reference simq(), returning integer grid + scale."""
    W = np.asarray(W, np.float32)
    m = np.max(np.abs(W))
    scale = (m / np.float32(QMAX)).astype(np.float32)
    q = np.clip(np.round(W / (scale + np.float32(1e-8))), -8, 7).astype(np.float32)
    return q, scale


def _simq_full(w):
    q, s = _simq_int(w)
    w = np.asarray(w, np.float32)
    if np.max(np.abs(w)) == 0:
        return w
    return (q * s).astype(np.float32)


def _bf(x):
    return np.ascontiguousarray(np.asarray(x, np.float32).astype(BF))


def _f32(x):
    return np.ascontiguousarray(np.asarray(x, np.float32))


def _pack_p(a, inner):
    """[D2, inner] -> [128, XT*inner], partition-major (row p holds all
    contraction-chunk slices contiguously)."""
    return np.ascontiguousarray(
        a.reshape(XT, 128, inner).transpose(1, 0, 2).reshape(128, XT * inner))


def _prepare(inputs):
    """Host input prep: quantize weights, fold scales, pack layouts.
    Returns (per-core in_maps, so*sv immediate)."""
    z_curr = np.asarray(inputs["z_curr_ri"], np.float32).reshape(B, D2)
    z_past = np.asarray(inputs["z_past_ri"], np.float32).reshape(P, B, D2)
    rel_bias = np.asarray(inputs["rel_bias_ri"], np.float32).reshape(-1, D2)
    curr_pos = int(np.asarray(inputs["curr_pos"]))

    Qk, sk = _simq_int(inputs["Wk"])
    Qv, sv = _simq_int(inputs["Wv"])
    Qq, sq = _simq_int(inputs["Wq"])
    Qo, so = _simq_int(inputs["Wo"])
    bk = _simq_full(inputs["bk"])
    bv = _simq_full(inputs["bv"])
    bq = _simq_full(inputs["bq"])
    bo = _simq_full(inputs["bo"])

    C_q = np.float32(sq * sk / math.sqrt(DIM // H))

    lbk_e = np.asarray(inputs["LBk"], np.float32).T * np.float32(4.0 / sk)  # [4, D2]
    lbv_e = np.asarray(inputs["LBv"], np.float32).T * np.float32(4.0 / sv)
    akv = np.concatenate([np.asarray(inputs["Ak"], np.float32).T,
                          np.asarray(inputs["Av"], np.float32).T], axis=1)  # [D2, 8]
    lbv128 = np.zeros((128, D2), np.float32)
    lbv128[RANK:2 * RANK] = lbv_e

    shared = {
        "wk": _bf(_pack_p(Qk.T, D2)), "wv": _bf(_pack_p(Qv.T, D2)),
        "wq": _bf(_pack_p(Qq.T, D2)), "wo": _bf(_pack_p(Qo.T, D2)),
        "aq": _bf(_pack_p(np.asarray(inputs["Aq"], np.float32).T, RANK)),
        "ao": _bf(_pack_p(np.asarray(inputs["Ao"], np.float32).T, RANK)),
        "akv": _bf(_pack_p(akv, 2 * RANK)),
        "lbkT": _bf(_pack_p(lbk_e.T, RANK)),
        "lbq": _bf(np.asarray(inputs["LBq"], np.float32).T * np.float32(4.0 / sq)),
        "lbo": _bf(np.asarray(inputs["LBo"], np.float32).T * np.float32(4.0 / so)),
        "lbv128": _bf(lbv128),
        "qb": _f32((bq * np.float32(sk / math.sqrt(DIM // H))).reshape(OT, 128).T),
        "vb": _f32((so * bv).reshape(OT, 128).T),
        "ob": _f32(bo.reshape(OT, 128).T),
    }
    idx = np.clip(np.arange(curr_pos - P, curr_pos) + REL_MAX, 0, rel_bias.shape[0] - 1)
    rbm = ((rel_bias[idx].T + bk[:, None]) / sk)  # [D2, P]
    shared["rb"] = _bf(rbm.reshape(OT, 128, CH // BL, CT).transpose(2, 1, 0, 3))

    in_maps = []
    for c in range(N_CORES):
        bsl = slice(c * BL, (c + 1) * BL)
        m = dict(shared)
        xpT = z_past[:, bsl, :].transpose(2, 1, 0).reshape(D2, NT)  # t = b*P + p
        m["xp"] = _bf(xpT.reshape(XT, 128, CH, CT).transpose(2, 1, 0, 3))
        m["xc"] = _bf(_pack_p(z_curr[bsl].T * C_q, BL))             # [128, XT*BL]
        in_maps.append(m)

    return in_maps, np.float32(so * sv)


def _build(so_sv, debug=False):
    """Build the per-core Bass/Tile program (identical on all 8 cores)."""
    import concourse.bass as bass
    import concourse.tile as tile
    from concourse import bacc, mybir
    from concourse.bass import ts
    from concourse.masks import make_identity

    f32 = mybir.dt.float32
    bf16 = mybir.dt.bfloat16
    PSUM = bass.MemorySpace.PSUM
    Exp = mybir.ActivationFunctionType.Exp
    AX = mybir.AxisListType.X
    ADD = mybir.AluOpType.add
    MULT = mybir.AluOpType.mult

    nc = bacc.Bacc("TRN2", target_bir_lowering=False, debug=False,
                   num_devices=N_CORES)

    def din(name, shape, dt=bf16):
        return nc.dram_tensor(name, shape, dt, kind="ExternalInput").ap()

    xp_d = din("xp", [CH, 128, XT, CT])
    xc_d = din("xc", [128, XT * BL])
    w_d = {n: din("w" + n, [128, XT * D2]) for n in "kvqo"}
    a_d = {n: din("a" + n, [128, XT * RANK]) for n in "qo"}
    lb_d = {n: din("lb" + n, [RANK, D2]) for n in "qo"}
    akv_d = din("akv", [128, XT * 2 * RANK])
    lbv128_d = din("lbv128", [128, D2])
    lbkT_d = din("lbkT", [128, OT * RANK])
    rb_d = din("rb", [CH // BL, 128, OT, CT])
    qb_d = din("qb", [128, OT], f32)
    vb_d = din("vb", [128, OT], f32)
    ob_d = din("ob", [128, OT], f32)
    out_d = nc.dram_tensor("out", [D2, BL], f32, kind="ExternalOutput").ap()
    if debug:
        dbg = {
            "d_qsm": nc.dram_tensor("d_qsm", [128, OT, BL], bf16, kind="ExternalOutput").ap(),
            "d_S": nc.dram_tensor("d_S", [BL * H, P], f32, kind="ExternalOutput").ap(),
            "d_wn": nc.dram_tensor("d_wn", [BL * H, P], bf16, kind="ExternalOutput").ap(),
            "d_rs": nc.dram_tensor("d_rs", [128, OT, BL], bf16, kind="ExternalOutput").ap(),
            "d_kt0": nc.dram_tensor("d_kt0", [128, CT], bf16, kind="ExternalOutput").ap(),
            "d_v0": nc.dram_tensor("d_v0", [128, D2], bf16, kind="ExternalOutput").ap(),
        }

    with tile.TileContext(nc) as tc, ExitStack() as ctx:
        consts = ctx.enter_context(tc.tile_pool(name="consts", bufs=1))
        res = ctx.enter_context(tc.tile_pool(name="res", bufs=1))

        # ---- resident loads (DMA emission order = startup critical path:
        # Q-phase deps first, then chunk-0 K deps, then V, then O-side) ----
        w_s = {}
        a_s = {}
        lb_s = {}

        def wload(n):
            w_s[n] = consts.tile([128, XT, D2], bf16, tag="w" + n, name="w" + n + "_s")
            nc.sync.dma_start(w_s[n][:], w_d[n].rearrange("p (c o) -> p c o", o=D2))

        def ablload(n):
            a_s[n] = consts.tile([128, XT, RANK], bf16, tag="a" + n, name="a" + n + "_s")
            nc.sync.dma_start(a_s[n][:], a_d[n].rearrange("p (c r) -> p c r", r=RANK))
            lb_s[n] = consts.tile([RANK, D2], bf16, tag="lb" + n, name="lb" + n + "_s")
            nc.sync.dma_start(lb_s[n][:], lb_d[n][:])

        xc_s = consts.tile([128, XT, BL], bf16, tag="xc")
        nc.sync.dma_start(xc_s[:], xc_d.rearrange("p (c b) -> p c b", b=BL))
        qb_s = consts.tile([128, OT], f32, tag="qb")
        nc.sync.dma_start(qb_s[:], qb_d[:])
        ablload("q")
        lbkT_s = consts.tile([128, OT, RANK], bf16, tag="lbkT")
        nc.sync.dma_start(lbkT_s[:], lbkT_d.rearrange("p (c r) -> p c r", r=RANK))
        wload("q")
        akv_s = consts.tile([128, XT, 2 * RANK], bf16, tag="akv")
        nc.sync.dma_start(akv_s[:], akv_d.rearrange("p (c r) -> p c r", r=2 * RANK))
        wload("k")
        lbv128_s = consts.tile([128, D2], bf16, tag="lbv128")
        nc.sync.dma_start(lbv128_s[:], lbv128_d[:])
        wload("v")
        wload("o")
        ablload("o")
        vb_s = consts.tile([128, OT], f32, tag="vb")
        nc.sync.dma_start(vb_s[:], vb_d[:])
        ob_s = consts.tile([128, OT], f32, tag="ob")
        nc.sync.dma_start(ob_s[:], ob_d[:])

        ident = consts.tile([BL * H, BL * H], bf16, tag="ident")
        make_identity(nc, ident[:])
        identf = consts.tile([H, H], f32, tag="identf")
        make_identity(nc, identf[:])

        vres = res.tile([128, NT // 128, D2], bf16, tag="vres")
        Sb = [res.tile([H, P], f32, tag=f"S{b}", name=f"S{b}") for b in range(BL)]
        q_sm = res.tile([128, OT, BL], bf16, tag="q_sm")
        wTb = [res.tile([128, PC, H], bf16, tag=f"wT{b}", name=f"wT{b}")
               for b in range(BL)]
        r_s = res.tile([128, OT, BL], bf16, tag="r_s")

        # ---- Q projection (tiny) ----
        with tc.tile_pool(name="pq_pool", bufs=2, space=PSUM) as pq_pool:
            t1q_p = pq_pool.tile([RANK, BL], f32, tag="t1q")
            for xc in range(XT):
                nc.tensor.matmul(t1q_p[:], a_s["q"][:, xc, :], xc_s[:, xc, :],
                                 start=(xc == 0), stop=(xc == XT - 1))
            t1q_s = res.tile([RANK, BL], bf16, tag="t1q_s")
            nc.vector.tensor_copy(t1q_s[:], t1q_p[:])
            for ot in range(OT):
                pq = pq_pool.tile([128, BL], f32, tag="pq")
                for xc in range(XT):
                    nc.tensor.matmul(pq[:], w_s["q"][:, xc, ts(ot, 128)],
                                     xc_s[:, xc, :], start=(xc == 0), stop=False)
                nc.tensor.matmul(pq[:], lb_s["q"][:, ts(ot, 128)], t1q_s[:],
                                 start=False, stop=True)
                nc.vector.tensor_scalar(q_sm[:, ot, :], pq[:],
                                        qb_s[:, ot:ot + 1], None, ADD)

        # zero-padded per-(batch, head) query slices: q_pad[b][:, h, :] is a
        # [128, 8] lhsT whose only nonzero column is h (holding q_sm[:, h, b])
        # -> per-chunk head matmuls accumulate into disjoint rows of an
        # [8, CT] PSUM tile with zero garbage.
        q_pad = []
        for b in range(BL):
            qp = res.tile([128, H, H], bf16, tag=f"q_pad{b}", name=f"q_pad{b}")
            nc.vector.memset(qp[:], 0.0)
            for h in range(H):
                nc.vector.tensor_copy(qp[:, h, h:h + 1], q_sm[:, h, b:b + 1])
            q_pad.append(qp)

        # qlb[b, h, r] = sum_o q_sm[o, b] * lbkT[o, r] over head-block h, packed
        # one-hot into [128, 8] lhsT tiles (rows RANK..128 zero) so the K-side
        # LoRA score term is one full-contraction matmul per chunk against the
        # zero-padded t1kv activations.
        qlb_pad = []
        for b in range(BL):
            qlp = res.tile([128, H], bf16, tag=f"qlb_pad{b}", name=f"qlb_pad{b}")
            nc.vector.memset(qlp[:], 0.0)
            qlb_pad.append(qlp)
        with tc.tile_pool(name="pqlb_pool", bufs=2, space=PSUM) as pqlb_pool:
            for h in range(H):
                pqlb = pqlb_pool.tile([BL, RANK], f32, tag="pqlb")
                nc.tensor.matmul(pqlb[:], q_sm[:, h, :], lbkT_s[:, h, :],
                                 start=True, stop=True)
                qlb_sb = res.tile([BL, RANK], bf16, tag="qlb_sb")
                nc.vector.tensor_copy(qlb_sb[:], pqlb[:])
                pqt = pqlb_pool.tile([RANK, BL], bf16, tag="pqt")
                nc.tensor.transpose(pqt[:], qlb_sb[:], ident[:BL, :BL])
                for b in range(BL):
                    nc.vector.tensor_copy(qlb_pad[b][0:RANK, h:h + 1],
                                          pqt[:, b:b + 1])

        # ---- main chunk loop: K production + scores + V^T production ----
        # batch-interleaved order so batch 0 finishes early and its softmax
        # overlaps the last chunk's compute
        order = [0, 4, 1, 5, 2, 6, 3, 7]
        with (
            tc.tile_pool(name="xt_pool", bufs=2) as xt_pool,
            tc.tile_pool(name="rb_pool", bufs=2) as rb_pool,
            tc.tile_pool(name="kt_pool", bufs=3) as kt_pool,
            tc.tile_pool(name="t1_pool", bufs=2) as t1_pool,
            tc.tile_pool(name="pkv_pool", bufs=5, space=PSUM) as pkv_pool,
            tc.tile_pool(name="ps_pool", bufs=2, space=PSUM) as ps_pool,
            tc.tile_pool(name="pt1_pool", bufs=1, space=PSUM) as pt1_pool,
        ):
            def softmax_b(b):
                mx = res.tile([H, 1], f32, tag="mx", name=f"mx{b}")
                nc.vector.reduce_max(mx[:], Sb[b][:], axis=AX)
                negmx = res.tile([H, 1], f32, tag="negmx", name=f"negmx{b}")
                nc.vector.tensor_scalar_mul(negmx[:], mx[:], -1.0)
                E = res.tile([H, P], bf16, tag="E", name=f"E{b}")
                nc.scalar.activation(E[:], Sb[b][:], Exp, bias=negmx[:], scale=1.0)
                lsum = res.tile([H, 1], f32, tag="lsum", name=f"lsum{b}")
                nc.vector.reduce_sum(lsum[:], E[:], axis=AX)
                linv = res.tile([H, 1], f32, tag="linv", name=f"linv{b}")
                nc.vector.reciprocal(linv[:], lsum[:])
                wn = res.tile([H, P], bf16, tag="wn", name=f"wn{b}")
                # fold so*sv into the softmax weights
                nc.vector.tensor_scalar(wn[:], E[:], linv[:], float(so_sv),
                                        MULT, MULT)
                return wn

            wns = [None, None]
            for ci, c in enumerate(order):
                b = c // (CH // BL)          # batch of this chunk
                pc0 = c % (CH // BL)         # position offset, units of CT
                xt = xt_pool.tile([128, XT, CT], bf16, tag="xt")
                nc.sync.dma_start(xt[:], xp_d[c])
                rbt = rb_pool.tile([128, OT, CT], bf16, tag="rbt")
                nc.sync.dma_start(rbt[:], rb_d[pc0])

                # merged K/V rank-4 LoRA activations, zero-padded to 128 rows
                # (keeps the PE weight-load pipeline unbroken downstream)
                t1kv_p = pt1_pool.tile([2 * RANK, CT], f32, tag="t1")
                for xc in range(XT):
                    nc.tensor.matmul(t1kv_p[:], akv_s[:, xc, :], xt[:, xc, :],
                                     start=(xc == 0), stop=(xc == XT - 1))
                t1kv_pad = t1_pool.tile([128, CT], bf16, tag="t1kv")
                nc.gpsimd.memset(t1kv_pad[:], 0.0)
                nc.vector.tensor_copy(t1kv_pad[0:2 * RANK, :], t1kv_p[:])

                # scores PSUM accumulator; first the K-LoRA term, then per-head
                # dot products, software-pipelined one head behind K production
                # so the rel-bias add (DVE) never stalls the PE.
                ps_S = ps_pool.tile([H, CT], f32, tag="ps")
                nc.tensor.matmul(ps_S[:], qlb_pad[b][:], t1kv_pad[:],
                                 start=True, stop=False)
                kts = [None] * OT
                for ot in range(OT):
                    pk = pkv_pool.tile([128, CT], f32, tag="pkv", name="pk")
                    for xc in range(XT):
                        nc.tensor.matmul(pk[:], w_s["k"][:, xc, ts(ot, 128)],
                                         xt[:, xc, :], start=(xc == 0),
                                         stop=(xc == XT - 1))
                    kt = kt_pool.tile([128, CT], bf16, tag="kt")
                    nc.vector.tensor_tensor(kt[:], pk[:], rbt[:, ot, :], ADD)
                    kts[ot] = kt
                    if ot > 0:
                        nc.tensor.matmul(ps_S[:], q_pad[b][:, ot - 1, :],
                                         kts[ot - 1][:], start=False, stop=False)
                    if debug and c == 0 and ot == 0:
                        nc.sync.dma_start(dbg["d_kt0"][:], kt[:])
                nc.tensor.matmul(ps_S[:], q_pad[b][:, OT - 1, :],
                                 kts[OT - 1][:], start=False, stop=True)
                nc.vector.tensor_copy(Sb[b][:, ts(pc0, CT)], ps_S[:])

                # V^T tiles -> resident
                for tt in range(CT // 128):
                    for oh in range(2):
                        pv = pkv_pool.tile([128, 512], f32, tag="pkv", name="pv")
                        for xc in range(XT):
                            nc.tensor.matmul(pv[:], xt[:, xc, ts(tt, 128)],
                                             w_s["v"][:, xc, ts(oh, 512)],
                                             start=(xc == 0), stop=False)
                        nc.tensor.matmul(pv[:], t1kv_pad[:, ts(tt, 128)],
                                         lbv128_s[:, ts(oh, 512)],
                                         start=False, stop=True)
                        nc.vector.tensor_copy(
                            vres[:, c * (CT // 128) + tt, ts(oh, 512)], pv[:])

                # batch 0 is fully scored after its last chunk: overlap its
                # softmax with the remaining chunk's compute
                if ci == CH - 2:
                    wns[0] = softmax_b(0)
            wns[1] = softmax_b(1)

        if debug:
            nc.sync.dma_start(dbg["d_qsm"][:], q_sm[:])
            nc.sync.dma_start(dbg["d_S"][0:H, :], Sb[0][:])
            nc.sync.dma_start(dbg["d_S"][H:2 * H, :], Sb[1][:])
            nc.sync.dma_start(dbg["d_v0"][:], vres[:, 0, :])
            nc.sync.dma_start(dbg["d_wn"][0:H, :], wns[0][:])
            nc.sync.dma_start(dbg["d_wn"][H:2 * H, :], wns[1][:])

        # ---- transpose softmax weights: [H, P] -> [P, H] per batch ----
        with tc.tile_pool(name="ptr_pool", bufs=2, space=PSUM) as ptr_pool:
            for b in range(BL):
                for pc in range(PC):
                    ptr = ptr_pool.tile([128, H], bf16, tag="ptr")
                    nc.tensor.transpose(ptr[:], wns[b][:, ts(pc, 128)],
                                        ident[:H, :H])
                    nc.vector.tensor_copy(wTb[b][:, pc, :], ptr[:])

        # ---- PV + output-side assembly (R = so*sv*PV + so*bv, [din, b]) ----
        with (
            tc.tile_pool(name="pacc_pool", bufs=4, space=PSUM) as pacc_pool,
            tc.tile_pool(name="ptr2_pool", bufs=2, space=PSUM) as ptr2_pool,
            tc.tile_pool(name="pvs_pool", bufs=2) as pvs_pool,
        ):
            for b in range(BL):
                for oh in range(2):
                    pacc = pacc_pool.tile([H, 512], f32, tag="pacc")
                    for pc in range(PC):
                        nc.tensor.matmul(pacc[:], wTb[b][:, pc, :],
                                         vres[:, b * PC + pc, ts(oh, 512)],
                                         start=(pc == 0), stop=(pc == PC - 1))
                    pvs = pvs_pool.tile([H, 512], f32, tag="pvs")
                    nc.vector.tensor_copy(pvs[:], pacc[:])
                    for hl in range(4):
                        h = oh * 4 + hl
                        ptr2 = ptr2_pool.tile([128, H], f32, tag="ptr2")
                        nc.tensor.transpose(ptr2[:], pvs[:, ts(hl, 128)],
                                            identf[:])
                        nc.vector.tensor_scalar(r_s[:, h, b:b + 1],
                                                ptr2[:, h:h + 1],
                                                vb_s[:, h:h + 1], None, ADD)

        if debug:
            nc.sync.dma_start(dbg["d_rs"][:], r_s[:])

        # ---- O projection ----
        with (
            tc.tile_pool(name="po_pool", bufs=2, space=PSUM) as po_pool,
            tc.tile_pool(name="pt1o_pool", bufs=1, space=PSUM) as pt1o_pool,
            tc.tile_pool(name="out_pool", bufs=2) as out_pool,
        ):
            t1o_p = pt1o_pool.tile([RANK, BL], f32, tag="t1o")
            for xc in range(XT):
                nc.tensor.matmul(t1o_p[:], a_s["o"][:, xc, :], r_s[:, xc, :],
                                 start=(xc == 0), stop=(xc == XT - 1))
            t1o_s = res.tile([RANK, BL], bf16, tag="t1o_s")
            nc.vector.tensor_copy(t1o_s[:], t1o_p[:])
            for ot in range(OT):
                po = po_pool.tile([128, BL], f32, tag="po")
                for xc in range(XT):
                    nc.tensor.matmul(po[:], w_s["o"][:, xc, ts(ot, 128)],
                                     r_s[:, xc, :], start=(xc == 0), stop=False)
                nc.tensor.matmul(po[:], lb_s["o"][:, ts(ot, 128)], t1o_s[:],
                                 start=False, stop=True)
                ot_s = out_pool.tile([128, BL], f32, tag="ot_s")
                nc.vector.tensor_scalar(ot_s[:], po[:], ob_s[:, ot:ot + 1],
                                        None, ADD)
                nc.sync.dma_start(out_d[ts(ot, 128), :], ot_s[:])

    nc.compile()
    return nc


def kernel(**inputs):
    global LAST_RESULT
    from concourse.bass_utils import run_bass_kernel_spmd

    in_maps, so_sv = _prepare(inputs)
    nc = _build(so_sv)
    res = run_bass_kernel_spmd(nc, in_maps, list(range(N_CORES)))
    LAST_RESULT = res
    outs = []
    for c in range(N_CORES):
        o = np.asarray(res.results[c]["out"], np.float32)  # [D2, BL]
        outs.append(o.T.reshape(BL, DIM, 2))
    return np.concatenate(outs, axis=0).astype(np.float32)
